# revision 3
# baseline (speedup 1.0000x reference)
"""GIN message-passing network on 8 Trainium2 NeuronCores — v4.

v1's structure (one AllGather of Z per layer, XT = Zfull^T @ A_T slice)
with the individually-validated improvements from v2/v3:

- `at` (transposed adjacency + (1+eps)I) in float8_e4m3: exact (small-int
  counts), halves its HBM traffic; matmul mixes fp8 rhs with bf16 lhsT.
- Hidden-layer Z runs m-outer so the layer's AllGather fires ~4us earlier.
- ApplyNodeFunc BN-ReLU + model BN-ReLU folded into ONE activation
  (s3>0, t3<=0 asserted on host).
- No trailing device AllReduce: per-core partial scores summed on host
  (part of unsharding), removing ~13us of tail latency.
- A tiny prelude AllGather absorbs part of the first-collective setup.
- All DMA on the sync ring, emitted in need-order; readout weight streams
  emitted behind the urgent compute streams.
"""

import numpy as np
import ml_dtypes

import concourse.bass as bass
import concourse.bacc as bacc
import concourse.tile as tile
import concourse.mybir as mybir
from concourse.bass_utils import run_bass_kernel_spmd

bf16 = ml_dtypes.bfloat16
fp8e4 = ml_dtypes.float8_e4m3
dt = mybir.dt
AF = mybir.ActivationFunctionType
ALU = mybir.AluOpType

N_FULL, H_FULL, C, NL, NCORES = 4096, 512, 2, 5, 8
NLAY = NL - 1  # 4 GIN layers


def build_program(N=N_FULL, H=H_FULL, ncores=NCORES):
    NPC = N // ncores          # nodes per core (512)
    KT0 = N // 128             # k-tiles over all nodes (32)
    HT = H // 128              # tiles over hidden dim (4)
    MT = NPC // 128            # m-tiles over this core's nodes (4)
    CH = 8                     # k-tiles per streamed DMA chunk
    NSLOT = KT0 + NLAY * HT    # readout accumulator slots per class

    nc = bacc.Bacc("TRN2", target_bir_lowering=False, debug=False,
                   num_devices=ncores)

    featT = nc.dram_tensor("featT", [N, NPC], dt.bfloat16, kind="ExternalInput")
    w0a = nc.dram_tensor("w0a", [N, H], dt.bfloat16, kind="ExternalInput")
    wra = nc.dram_tensor("wra", [NLAY - 1, H, H], dt.bfloat16, kind="ExternalInput")
    wb = nc.dram_tensor("wb", [NLAY, H, H], dt.bfloat16, kind="ExternalInput")
    at = nc.dram_tensor("at", [NLAY, N, NPC], dt.float8e4, kind="ExternalInput")
    wp0 = nc.dram_tensor("wp0", [C, N, NPC], dt.bfloat16, kind="ExternalInput")
    wpr = nc.dram_tensor("wpr", [NLAY, C, H, NPC], dt.bfloat16, kind="ExternalInput")
    aff = nc.dram_tensor("aff", [128, NLAY * 4 * HT], dt.float32, kind="ExternalInput")
    score = nc.dram_tensor("score", [C, 1], dt.float32, kind="ExternalOutput")

    rg = [list(range(ncores))]

    def aff_col(lay, stage, m):
        return lay * 4 * HT + stage * HT + m

    with tile.TileContext(nc) as tc:
        with (
            tc.tile_pool(name="dram", bufs=2, space="DRAM") as dram,
            tc.tile_pool(name="big", bufs=1) as big,
            tc.tile_pool(name="sb", bufs=2) as sb,
            tc.tile_pool(name="stream", bufs=4) as stream,
            tc.tile_pool(name="acc", bufs=8, space="PSUM") as psum,
        ):
            # ---- prelude dummy AllGather ----
            dsb = sb.tile([2, 1], dt.float32, tag="dsb")
            nc.vector.memset(dsb[:], 0.0)
            dbin = dram.tile([2, 1], dt.float32, tag="dbin")
            nc.sync.dma_start(dbin[:], dsb[:])
            dbout = dram.tile([2 * ncores, 1], dt.float32, tag="dbout",
                              addr_space="Shared")
            nc.gpsimd.collective_compute(
                "AllGather", ALU.bypass, replica_groups=rg,
                ins=[dbin.opt()], outs=[dbout.opt()])
            # dback is read at the TAIL (emitting its DMA here would
            # head-of-line-block every head load behind the dummy AllGather)
            dback = sb.tile([2, 1], dt.float32, tag="dback")

            # ---- resident constants ----
            aff_sb = big.tile([128, NLAY * 4 * HT], dt.float32, tag="aff")
            nc.sync.dma_start(aff_sb[:], aff[:])
            racc = big.tile([128, C * NSLOT], dt.float32, tag="racc")

            featT_sb = big.tile([128, KT0, NPC], dt.bfloat16, tag="featT")
            wra_sb = big.tile([128, (NLAY - 1) * HT, H], dt.bfloat16, tag="wra")
            wb_sb = big.tile([128, NLAY * HT, H], dt.bfloat16, tag="wb")

            w0a_t = []
            for k0 in range(0, KT0, CH):
                nc.sync.dma_start(
                    featT_sb[:, k0:k0 + CH, :],
                    featT[k0 * 128:(k0 + CH) * 128, :].rearrange("(t p) h -> p t h", p=128))
                wt = stream.tile([128, CH, H], dt.bfloat16, tag="wa", bufs=4,
                                 name=f"w0a{k0}")
                nc.sync.dma_start(
                    wt[:],
                    w0a[k0 * 128:(k0 + CH) * 128, :].rearrange("(t p) h -> p t h", p=128))
                w0a_t.append(wt)
            nc.sync.dma_start(
                wb_sb[:, 0:HT, :], wb[0].rearrange("(t p) h -> p t h", p=128))

            # ---- building blocks ----
            def z_evac_ag(psZ):
                """psZ m-tiles -> bf16 -> DRAM -> single AllGather."""
                zcat = stream.tile([128, MT, H], dt.bfloat16, tag="zcat", bufs=2)
                for m in range(MT):
                    nc.vector.tensor_copy(zcat[:, m, :], psZ[m][:])
                zin = dram.tile([NPC, H], dt.bfloat16, tag="zin", bufs=2)
                nc.sync.dma_start(
                    zin.rearrange("(m p) h -> p m h", p=128), zcat[:])
                zfull = dram.tile([N, H], dt.bfloat16, tag="zfull", bufs=2,
                                  addr_space="Shared")
                nc.gpsimd.collective_compute(
                    "AllGather", ALU.bypass, replica_groups=rg,
                    ins=[zin.opt()], outs=[zfull.opt()])
                return zfull

            def xt_block(lay, zfull):
                """XT_c = Zfull^T @ (A_T+(1+eps)I)_c -> [H, NPC].

                All `at` chunk loads are emitted BEFORE the zf loads: zf
                chunk 0 waits on the AllGather, and a waiting DMA blocks the
                sync ring behind it — at must already be in flight."""
                psX = [psum.tile([128, NPC], dt.float32, tag="acc",
                                 name=f"psX{lay}_{m}") for m in range(HT)]
                at_ts = []
                for k0 in range(0, KT0, CH):
                    at_t = stream.tile([128, CH, NPC], dt.float8e4, tag="at", bufs=4)
                    nc.sync.dma_start(
                        at_t[:],
                        at[lay, k0 * 128:(k0 + CH) * 128, :]
                        .rearrange("(t p) h -> p t h", p=128))
                    at_ts.append(at_t)
                # graded zf chunks: XT starts after the first 0.25 MB lands
                # instead of waiting for a full 1 MB chunk.
                k0 = 0
                for cc in (2, 6, CH, CH, CH):
                    zf = stream.tile([128, CH, H], dt.bfloat16, tag="zf", bufs=3)
                    nc.sync.dma_start(
                        zf[:, 0:cc, :],
                        zfull[k0 * 128:(k0 + cc) * 128, :]
                        .rearrange("(t p) h -> p t h", p=128))
                    for kk in range(cc):
                        k = k0 + kk
                        for m in range(HT):
                            nc.tensor.matmul(
                                psX[m][:], lhsT=zf[:, kk, m * 128:(m + 1) * 128],
                                rhs=at_ts[k // CH][:, k % CH, :],
                                start=(k == 0), stop=(k == KT0 - 1))
                    k0 += cc
                assert k0 == KT0
                return psX

            def mlp_tail(lay, psX):
                """act1 interleaved with YT, then fused act23 -> hT (bf16)."""
                xt_sb = sb.tile([128, HT, NPC], dt.bfloat16, tag="xt")
                psY = [psum.tile([128, NPC], dt.float32, tag="acc",
                                 name=f"psY{lay}_{m}") for m in range(HT)]
                for k in range(HT):
                    nc.scalar.activation(
                        xt_sb[:, k, :], psX[k][:], AF.Relu,
                        bias=aff_sb[:, aff_col(lay, 1, k):aff_col(lay, 1, k) + 1],
                        scale=aff_sb[:, aff_col(lay, 0, k):aff_col(lay, 0, k) + 1])
                    for m in range(HT):
                        nc.tensor.matmul(
                            psY[m][:], lhsT=wb_sb[:, lay * HT + k, m * 128:(m + 1) * 128],
                            rhs=xt_sb[:, k, :], start=(k == 0), stop=(k == HT - 1))
                hT_sb = sb.tile([128, HT, NPC], dt.bfloat16, tag="hT")
                for m in range(HT):
                    nc.scalar.activation(
                        hT_sb[:, m, :], psY[m][:], AF.Relu,
                        bias=aff_sb[:, aff_col(lay, 3, m):aff_col(lay, 3, m) + 1],
                        scale=aff_sb[:, aff_col(lay, 2, m):aff_col(lay, 2, m) + 1])
                return hT_sb

            def emit_feat_readout_quarter(q):
                KPC = KT0 // 4
                for c in range(C):
                    wt = stream.tile([128, KPC, NPC], dt.bfloat16, tag="wro", bufs=2)
                    nc.sync.dma_start(
                        wt[:],
                        wp0[c, q * KPC * 128:(q + 1) * KPC * 128, :]
                        .rearrange("(t p) h -> p t h", p=128))
                    for kk in range(KPC):
                        k = q * KPC + kk
                        scr = stream.tile([128, NPC], dt.float32, tag="scr", bufs=2)
                        nc.vector.scalar_tensor_tensor(
                            out=scr[:], in0=featT_sb[:, k, :], scalar=1.0,
                            in1=wt[:, kk, :], op0=ALU.mult, op1=ALU.mult,
                            accum_out=racc[:, c * NSLOT + k: c * NSLOT + k + 1])

            def emit_h_readout(lay, hT_sb):
                for c in range(C):
                    wt = stream.tile([128, HT, NPC], dt.bfloat16, tag="wrr", bufs=2)
                    nc.sync.dma_start(
                        wt[:], wpr[lay, c].rearrange("(t p) h -> p t h", p=128))
                    for m in range(HT):
                        scr = stream.tile([128, NPC], dt.float32, tag="scr", bufs=2)
                        slot = c * NSLOT + KT0 + lay * HT + m
                        nc.vector.scalar_tensor_tensor(
                            out=scr[:], in0=hT_sb[:, m, :], scalar=1.0,
                            in1=wt[:, m, :], op0=ALU.mult, op1=ALU.mult,
                            accum_out=racc[:, slot:slot + 1])

            # ================= layer 0 =================
            psZ = [psum.tile([128, H], dt.float32, tag="acc", name=f"psZ0_{m}")
                   for m in range(MT)]
            for k in range(KT0):
                for m in range(MT):
                    nc.tensor.matmul(
                        psZ[m][:], lhsT=featT_sb[:, k, m * 128:(m + 1) * 128],
                        rhs=w0a_t[k // CH][:, k % CH, :],
                        start=(k == 0), stop=(k == KT0 - 1))
            zfull = z_evac_ag(psZ)
            psX = xt_block(0, zfull)
            # resident weights for later layers stream behind layer-0 XT
            for l in range(NLAY - 1):
                nc.sync.dma_start(
                    wra_sb[:, l * HT:(l + 1) * HT, :],
                    wra[l].rearrange("(t p) h -> p t h", p=128))
            for l in range(1, NLAY):
                nc.sync.dma_start(
                    wb_sb[:, l * HT:(l + 1) * HT, :],
                    wb[l].rearrange("(t p) h -> p t h", p=128))
            emit_feat_readout_quarter(0)
            hT = mlp_tail(0, psX)

            # ================= layers 1..3 =================
            for lay in range(1, NLAY):
                # Z m-outer: evac+AllGather fire as soon as all m done
                psZ = [psum.tile([128, H], dt.float32, tag="acc",
                                 name=f"psZ{lay}_{m}") for m in range(MT)]
                for m in range(MT):
                    for k in range(HT):
                        nc.tensor.matmul(
                            psZ[m][:], lhsT=hT[:, k, m * 128:(m + 1) * 128],
                            rhs=wra_sb[:, (lay - 1) * HT + k, :],
                            start=(k == 0), stop=(k == HT - 1))
                zfull = z_evac_ag(psZ)
                hT_prev = hT
                psX = xt_block(lay, zfull)
                emit_h_readout(lay - 1, hT_prev)
                emit_feat_readout_quarter(lay)
                hT = mlp_tail(lay, psX)

            # ================= tail =================
            emit_h_readout(NLAY - 1, hT)
            r2 = sb.tile([128, C], dt.float32, tag="r2")
            for c in range(C):
                nc.vector.tensor_reduce(
                    r2[:, c:c + 1], racc[:, c * NSLOT:(c + 1) * NSLOT],
                    axis=mybir.AxisListType.X, op=ALU.add)
            ones = sb.tile([128, 1], dt.float32, tag="ones")
            nc.vector.memset(ones[:], 1.0)
            nc.sync.dma_start(dback[:], dbout[0:2, :])
            psS = psum.tile([C, 1], dt.float32, tag="acc")
            nc.tensor.matmul(psS[:], lhsT=r2[:], rhs=ones[:], start=True, stop=True)
            o_sb = sb.tile([C, 1], dt.float32, tag="o_sb")
            nc.vector.scalar_tensor_tensor(
                out=o_sb[:], in0=dback[:], scalar=0.0, in1=psS[:],
                op0=ALU.mult, op1=ALU.add)
            nc.sync.dma_start(score[:], o_sb[:])

    nc.compile()
    return nc


def prep_inputs(inputs, N=N_FULL, H=H_FULL, ncores=NCORES, nlay=NLAY):
    """Host-side re-layout of the full inputs into per-core input maps."""
    inp = {k: np.asarray(v) for k, v in inputs.items()}
    NPC = N // ncores
    HT = H // 128
    f32 = np.float32

    feat = inp["feat"].astype(f32)
    src = inp["edge_src"].astype(np.int64)
    dst = inp["edge_dst"].astype(np.int64)

    A_T = np.zeros((N, N), f32)
    np.add.at(A_T, (src, dst), 1.0)
    eps_list = [float(inp["eps0"])] + [float(x) for x in inp["epsR"]]
    diag = np.arange(N)
    at_all = np.empty((nlay, N, N), fp8e4)
    for i in range(nlay):
        M = A_T.copy()
        M[diag, diag] += 1.0 + eps_list[i]
        at_all[i] = M.astype(fp8e4)

    featT = np.ascontiguousarray(feat.T).astype(bf16)
    w0a = inp["W0a"].astype(f32).astype(bf16)
    wra = inp["WRa"].astype(f32).astype(bf16)
    wb = np.concatenate([inp["W0b"][None], inp["WRb"]], axis=0).astype(f32).astype(bf16)

    ba = [inp["b0a"]] + [inp["bRa"][i] for i in range(nlay - 1)]
    bb = [inp["b0b"]] + [inp["bRb"][i] for i in range(nlay - 1)]

    def fold(nm, i):
        idx = (lambda x: x) if i == 0 else (lambda x: x[i - 1])
        g, b_, m, v = [idx(inp[nm + s]) for s in ("_g", "_b", "_m", "_v")]
        s = (g / np.sqrt(v + 1e-5)).astype(f32)
        return s, b_, m

    aff = np.zeros((128, nlay * 4 * HT), f32)
    for i in range(nlay):
        nms = ("bn0a", "bnA0", "bnO0") if i == 0 else ("bnRa", "bnAR", "bnOR")
        s, b_, m = fold(nms[0], i)
        p1s, p1t = s, ((ba[i] - m) * s + b_).astype(f32)
        s, b_, m = fold(nms[1], i)
        p2s, p2t = s, ((bb[i] - m) * s + b_).astype(f32)
        s, b_, m = fold(nms[2], i)
        p3s, p3t = s, (b_ - m * s).astype(f32)
        assert np.all(p3s > 0) and np.all(p3t <= 0), "act23 fold invalid"
        p23s, p23t = (p2s * p3s).astype(f32), (p2t * p3s + p3t).astype(f32)
        for mi in range(HT):
            sl = slice(mi * 128, (mi + 1) * 128)
            for j, vec in enumerate((p1s, p1t, p23s, p23t)):
                aff[:, i * 4 * HT + j * HT + mi] = vec[sl]

    wp0r = np.ascontiguousarray(
        inp["Wp0"].astype(f32).reshape(N, N, C).transpose(2, 1, 0)).astype(bf16)
    wprr = np.ascontiguousarray(
        inp["WpR"].astype(f32).reshape(nlay, N, H, C).transpose(0, 3, 2, 1)).astype(bf16)

    in_maps = []
    for cix in range(ncores):
        sl = slice(cix * NPC, (cix + 1) * NPC)
        in_maps.append({
            "featT": np.ascontiguousarray(featT[:, sl]),
            "w0a": w0a,
            "wra": wra,
            "wb": wb,
            "at": np.ascontiguousarray(at_all[:, :, sl]),
            "wp0": np.ascontiguousarray(wp0r[:, :, sl]),
            "wpr": np.ascontiguousarray(wprr[:, :, :, sl]),
            "aff": aff,
        })
    return in_maps


def host_bias(inputs):
    inp = {k: np.asarray(v) for k, v in inputs.items()}
    return (inp["bp0"] + inp["bpR"].sum(axis=0)).astype(np.float32).reshape(1, C)


_CACHE = {}


def _get_program():
    if "nc" not in _CACHE:
        _CACHE["nc"] = build_program()
    return _CACHE["nc"]


def kernel(**inputs):
    nc = _get_program()
    in_maps = prep_inputs(inputs)
    res = run_bass_kernel_spmd(nc, in_maps, list(range(NCORES)))
    total = np.zeros((1, C), np.float32)
    for r in res.results:
        total += np.asarray(r["score"], np.float32).reshape(1, C)
    return (total + host_bias(inputs)).astype(np.float32)


# revision 5
# speedup vs baseline: 1.9974x; 1.9974x over previous
"""GIN message-passing network on 8 Trainium2 NeuronCores — v4.

v1's structure (one AllGather of Z per layer, XT = Zfull^T @ A_T slice)
with the individually-validated improvements from v2/v3:

- `at` (transposed adjacency + (1+eps)I) in float8_e4m3: exact (small-int
  counts), halves its HBM traffic; matmul mixes fp8 rhs with bf16 lhsT.
- Hidden-layer Z runs m-outer so the layer's AllGather fires ~4us earlier.
- ApplyNodeFunc BN-ReLU + model BN-ReLU folded into ONE activation
  (s3>0, t3<=0 asserted on host).
- No trailing device AllReduce: per-core partial scores summed on host
  (part of unsharding), removing ~13us of tail latency.
- A tiny prelude AllGather absorbs part of the first-collective setup.
- All DMA on the sync ring, emitted in need-order; readout weight streams
  emitted behind the urgent compute streams.
"""

import numpy as np
import ml_dtypes

import concourse.bass as bass
import concourse.bacc as bacc
import concourse.tile as tile
import concourse.mybir as mybir
from concourse.bass_utils import run_bass_kernel_spmd

bf16 = ml_dtypes.bfloat16
fp8e4 = ml_dtypes.float8_e4m3
dt = mybir.dt
AF = mybir.ActivationFunctionType
ALU = mybir.AluOpType

N_FULL, H_FULL, C, NL, NCORES = 4096, 512, 2, 5, 8
NLAY = NL - 1  # 4 GIN layers


def build_program(N=N_FULL, H=H_FULL, ncores=NCORES):
    NPC = N // ncores          # nodes per core (512)
    KT0 = N // 128             # k-tiles over all nodes (32)
    HT = H // 128              # tiles over hidden dim (4)
    MT = NPC // 128            # m-tiles over this core's nodes (4)
    CH = 8                     # k-tiles per streamed DMA chunk
    NSLOT = KT0 + NLAY * HT    # readout accumulator slots per class

    nc = bacc.Bacc("TRN2", target_bir_lowering=False, debug=False,
                   num_devices=ncores)

    # featT/w0a are host-pretiled to partition-major [128, KT0, *] so the
    # head-critical DMAs are contiguous 8KB-per-partition lines.
    featT = nc.dram_tensor("featT", [128, KT0, NPC], dt.bfloat16, kind="ExternalInput")
    w0a = nc.dram_tensor("w0a", [128, KT0, H], dt.bfloat16, kind="ExternalInput")
    wra = nc.dram_tensor("wra", [NLAY - 1, H, H], dt.bfloat16, kind="ExternalInput")
    wb = nc.dram_tensor("wb", [NLAY, H, H], dt.bfloat16, kind="ExternalInput")
    at = nc.dram_tensor("at", [NLAY, N, NPC], dt.float8e4, kind="ExternalInput")
    wp0 = nc.dram_tensor("wp0", [C, N, NPC], dt.bfloat16, kind="ExternalInput")
    wpr = nc.dram_tensor("wpr", [NLAY, C, H, NPC], dt.bfloat16, kind="ExternalInput")
    aff = nc.dram_tensor("aff", [128, NLAY * 4 * HT], dt.float32, kind="ExternalInput")
    score = nc.dram_tensor("score", [C, 1], dt.float32, kind="ExternalOutput")

    rg = [list(range(ncores))]

    def aff_col(lay, stage, m):
        return lay * 4 * HT + stage * HT + m

    with tile.TileContext(nc) as tc:
        with (
            tc.tile_pool(name="dram", bufs=2, space="DRAM") as dram,
            tc.tile_pool(name="big", bufs=1) as big,
            tc.tile_pool(name="sb", bufs=2) as sb,
            tc.tile_pool(name="stream", bufs=4) as stream,
            tc.tile_pool(name="acc", bufs=8, space="PSUM") as psum,
        ):
            # ---- resident constants ----
            aff_sb = big.tile([128, NLAY * 4 * HT], dt.float32, tag="aff")
            racc = big.tile([128, C * NSLOT], dt.float32, tag="racc")

            featT_sb = big.tile([128, KT0, NPC], dt.bfloat16, tag="featT")
            wra_sb = big.tile([128, (NLAY - 1) * HT, H], dt.bfloat16, tag="wra")
            wb_sb = big.tile([128, NLAY * HT, H], dt.bfloat16, tag="wb")

            # graded head chunks: the first Z matmul starts after 128KB
            # instead of 1MB of featT/w0a.
            HEAD_CC = (1, 3, 4, CH, CH, CH)
            w0a_t = []     # (tile, chunk_start) per chunk
            k0 = 0
            for cc in HEAD_CC:
                nc.sync.dma_start(
                    featT_sb[:, k0:k0 + cc, :], featT[:, k0:k0 + cc, :])
                wt = stream.tile([128, CH, H], dt.bfloat16, tag="wa", bufs=6,
                                 name=f"w0a{k0}")
                nc.sync.dma_start(wt[:, 0:cc, :], w0a[:, k0:k0 + cc, :])
                w0a_t.append((wt, k0))
                k0 += cc
            assert k0 == KT0
            # aff is only needed by layer-0's act1, ~100us in — keep it off
            # the head-critical ring position.
            nc.sync.dma_start(aff_sb[:], aff[:])
            nc.sync.dma_start(
                wb_sb[:, 0:HT, :], wb[0].rearrange("(t p) h -> p t h", p=128))

            # ---- building blocks ----
            def z_evac_ag(psZ):
                """psZ m-tiles -> bf16 -> DRAM -> single AllGather."""
                zcat = stream.tile([128, MT, H], dt.bfloat16, tag="zcat", bufs=2)
                for m in range(MT):
                    nc.vector.tensor_copy(zcat[:, m, :], psZ[m][:])
                zin = dram.tile([NPC, H], dt.bfloat16, tag="zin", bufs=2)
                nc.sync.dma_start(
                    zin.rearrange("(m p) h -> p m h", p=128), zcat[:])
                zfull = dram.tile([N, H], dt.bfloat16, tag="zfull", bufs=2,
                                  addr_space="Shared")
                nc.gpsimd.collective_compute(
                    "AllGather", ALU.bypass, replica_groups=rg,
                    ins=[zin.opt()], outs=[zfull.opt()])
                return zfull

            def xt_block(lay, zfull):
                """XT_c = Zfull^T @ (A_T+(1+eps)I)_c -> [H, NPC].

                All `at` chunk loads are emitted BEFORE the zf loads: zf
                chunk 0 waits on the AllGather, and a waiting DMA blocks the
                sync ring behind it — at must already be in flight."""
                psX = [psum.tile([128, NPC], dt.float32, tag="acc",
                                 name=f"psX{lay}_{m}") for m in range(HT)]
                at_ts = []
                for k0 in range(0, KT0, CH):
                    at_t = stream.tile([128, CH, NPC], dt.float8e4, tag="at", bufs=4)
                    nc.sync.dma_start(
                        at_t[:],
                        at[lay, k0 * 128:(k0 + CH) * 128, :]
                        .rearrange("(t p) h -> p t h", p=128))
                    at_ts.append(at_t)
                # graded zf chunks: XT starts after the first 0.25 MB lands
                # instead of waiting for a full 1 MB chunk.
                k0 = 0
                for cc in (1, 3, 4, CH, CH, CH):
                    zf = stream.tile([128, CH, H], dt.bfloat16, tag="zf", bufs=3)
                    nc.sync.dma_start(
                        zf[:, 0:cc, :],
                        zfull[k0 * 128:(k0 + cc) * 128, :]
                        .rearrange("(t p) h -> p t h", p=128))
                    for kk in range(cc):
                        k = k0 + kk
                        for m in range(HT):
                            nc.tensor.matmul(
                                psX[m][:], lhsT=zf[:, kk, m * 128:(m + 1) * 128],
                                rhs=at_ts[k // CH][:, k % CH, :],
                                start=(k == 0), stop=(k == KT0 - 1))
                    k0 += cc
                assert k0 == KT0
                return psX

            def mlp_tail(lay, psX):
                """act1 interleaved with YT, then fused act23 -> hT (bf16)."""
                xt_sb = sb.tile([128, HT, NPC], dt.bfloat16, tag="xt")
                psY = [psum.tile([128, NPC], dt.float32, tag="acc",
                                 name=f"psY{lay}_{m}") for m in range(HT)]
                for k in range(HT):
                    nc.scalar.activation(
                        xt_sb[:, k, :], psX[k][:], AF.Relu,
                        bias=aff_sb[:, aff_col(lay, 1, k):aff_col(lay, 1, k) + 1],
                        scale=aff_sb[:, aff_col(lay, 0, k):aff_col(lay, 0, k) + 1])
                    for m in range(HT):
                        nc.tensor.matmul(
                            psY[m][:], lhsT=wb_sb[:, lay * HT + k, m * 128:(m + 1) * 128],
                            rhs=xt_sb[:, k, :], start=(k == 0), stop=(k == HT - 1))
                hT_sb = sb.tile([128, HT, NPC], dt.bfloat16, tag="hT")
                for m in range(HT):
                    nc.scalar.activation(
                        hT_sb[:, m, :], psY[m][:], AF.Relu,
                        bias=aff_sb[:, aff_col(lay, 3, m):aff_col(lay, 3, m) + 1],
                        scale=aff_sb[:, aff_col(lay, 2, m):aff_col(lay, 2, m) + 1])
                return hT_sb

            def emit_feat_readout_quarter(q):
                KPC = KT0 // 4
                for c in range(C):
                    wt = stream.tile([128, KPC, NPC], dt.bfloat16, tag="wro", bufs=2)
                    nc.sync.dma_start(
                        wt[:],
                        wp0[c, q * KPC * 128:(q + 1) * KPC * 128, :]
                        .rearrange("(t p) h -> p t h", p=128))
                    for kk in range(KPC):
                        k = q * KPC + kk
                        scr = stream.tile([128, NPC], dt.float32, tag="scr", bufs=2)
                        nc.vector.scalar_tensor_tensor(
                            out=scr[:], in0=featT_sb[:, k, :], scalar=1.0,
                            in1=wt[:, kk, :], op0=ALU.mult, op1=ALU.mult,
                            accum_out=racc[:, c * NSLOT + k: c * NSLOT + k + 1])

            def emit_h_readout(lay, hT_sb):
                for c in range(C):
                    wt = stream.tile([128, HT, NPC], dt.bfloat16, tag="wrr", bufs=2)
                    nc.sync.dma_start(
                        wt[:], wpr[lay, c].rearrange("(t p) h -> p t h", p=128))
                    for m in range(HT):
                        scr = stream.tile([128, NPC], dt.float32, tag="scr", bufs=2)
                        slot = c * NSLOT + KT0 + lay * HT + m
                        nc.vector.scalar_tensor_tensor(
                            out=scr[:], in0=hT_sb[:, m, :], scalar=1.0,
                            in1=wt[:, m, :], op0=ALU.mult, op1=ALU.mult,
                            accum_out=racc[:, slot:slot + 1])

            # ================= layer 0 =================
            k_to_chunk = []
            for ci, cc in enumerate(HEAD_CC):
                k_to_chunk += [ci] * cc
            psZ = [psum.tile([128, H], dt.float32, tag="acc", name=f"psZ0_{m}")
                   for m in range(MT)]
            for k in range(KT0):
                wt, cstart = w0a_t[k_to_chunk[k]]
                for m in range(MT):
                    nc.tensor.matmul(
                        psZ[m][:], lhsT=featT_sb[:, k, m * 128:(m + 1) * 128],
                        rhs=wt[:, k - cstart, :],
                        start=(k == 0), stop=(k == KT0 - 1))
            zfull = z_evac_ag(psZ)
            psX = xt_block(0, zfull)
            # resident weights for later layers stream behind layer-0 XT
            for l in range(NLAY - 1):
                nc.sync.dma_start(
                    wra_sb[:, l * HT:(l + 1) * HT, :],
                    wra[l].rearrange("(t p) h -> p t h", p=128))
            for l in range(1, NLAY):
                nc.sync.dma_start(
                    wb_sb[:, l * HT:(l + 1) * HT, :],
                    wb[l].rearrange("(t p) h -> p t h", p=128))
            emit_feat_readout_quarter(0)
            hT = mlp_tail(0, psX)

            # ================= layers 1..3 =================
            for lay in range(1, NLAY):
                # Z m-outer: evac+AllGather fire as soon as all m done
                psZ = [psum.tile([128, H], dt.float32, tag="acc",
                                 name=f"psZ{lay}_{m}") for m in range(MT)]
                for m in range(MT):
                    for k in range(HT):
                        nc.tensor.matmul(
                            psZ[m][:], lhsT=hT[:, k, m * 128:(m + 1) * 128],
                            rhs=wra_sb[:, (lay - 1) * HT + k, :],
                            start=(k == 0), stop=(k == HT - 1))
                zfull = z_evac_ag(psZ)
                hT_prev = hT
                psX = xt_block(lay, zfull)
                emit_h_readout(lay - 1, hT_prev)
                emit_feat_readout_quarter(lay)
                hT = mlp_tail(lay, psX)

            # ================= tail =================
            emit_h_readout(NLAY - 1, hT)
            r2 = sb.tile([128, C], dt.float32, tag="r2")
            for c in range(C):
                nc.vector.tensor_reduce(
                    r2[:, c:c + 1], racc[:, c * NSLOT:(c + 1) * NSLOT],
                    axis=mybir.AxisListType.X, op=ALU.add)
            ones = sb.tile([128, 1], dt.float32, tag="ones")
            nc.vector.memset(ones[:], 1.0)
            psS = psum.tile([C, 1], dt.float32, tag="acc")
            nc.tensor.matmul(psS[:], lhsT=r2[:], rhs=ones[:], start=True, stop=True)
            o_sb = sb.tile([C, 1], dt.float32, tag="o_sb")
            nc.vector.tensor_copy(o_sb[:], psS[:])
            nc.sync.dma_start(score[:], o_sb[:])

    nc.compile()
    return nc


def prep_inputs(inputs, N=N_FULL, H=H_FULL, ncores=NCORES, nlay=NLAY):
    """Host-side re-layout of the full inputs into per-core input maps."""
    inp = {k: np.asarray(v) for k, v in inputs.items()}
    NPC = N // ncores
    HT = H // 128
    f32 = np.float32

    feat = inp["feat"].astype(f32)
    src = inp["edge_src"].astype(np.int64)
    dst = inp["edge_dst"].astype(np.int64)

    A_T = np.zeros((N, N), f32)
    np.add.at(A_T, (src, dst), 1.0)
    eps_list = [float(inp["eps0"])] + [float(x) for x in inp["epsR"]]
    diag = np.arange(N)
    at_all = np.empty((nlay, N, N), fp8e4)
    for i in range(nlay):
        M = A_T.copy()
        M[diag, diag] += 1.0 + eps_list[i]
        at_all[i] = M.astype(fp8e4)

    KT0 = N // 128
    featT = np.ascontiguousarray(feat.T).astype(bf16)
    w0a_tiled = np.ascontiguousarray(
        inp["W0a"].astype(f32).astype(bf16).reshape(KT0, 128, H).transpose(1, 0, 2))
    wra = inp["WRa"].astype(f32).astype(bf16)
    wb = np.concatenate([inp["W0b"][None], inp["WRb"]], axis=0).astype(f32).astype(bf16)

    ba = [inp["b0a"]] + [inp["bRa"][i] for i in range(nlay - 1)]
    bb = [inp["b0b"]] + [inp["bRb"][i] for i in range(nlay - 1)]

    def fold(nm, i):
        idx = (lambda x: x) if i == 0 else (lambda x: x[i - 1])
        g, b_, m, v = [idx(inp[nm + s]) for s in ("_g", "_b", "_m", "_v")]
        s = (g / np.sqrt(v + 1e-5)).astype(f32)
        return s, b_, m

    aff = np.zeros((128, nlay * 4 * HT), f32)
    for i in range(nlay):
        nms = ("bn0a", "bnA0", "bnO0") if i == 0 else ("bnRa", "bnAR", "bnOR")
        s, b_, m = fold(nms[0], i)
        p1s, p1t = s, ((ba[i] - m) * s + b_).astype(f32)
        s, b_, m = fold(nms[1], i)
        p2s, p2t = s, ((bb[i] - m) * s + b_).astype(f32)
        s, b_, m = fold(nms[2], i)
        p3s, p3t = s, (b_ - m * s).astype(f32)
        assert np.all(p3s > 0) and np.all(p3t <= 0), "act23 fold invalid"
        p23s, p23t = (p2s * p3s).astype(f32), (p2t * p3s + p3t).astype(f32)
        for mi in range(HT):
            sl = slice(mi * 128, (mi + 1) * 128)
            for j, vec in enumerate((p1s, p1t, p23s, p23t)):
                aff[:, i * 4 * HT + j * HT + mi] = vec[sl]

    wp0r = np.ascontiguousarray(
        inp["Wp0"].astype(f32).reshape(N, N, C).transpose(2, 1, 0)).astype(bf16)
    wprr = np.ascontiguousarray(
        inp["WpR"].astype(f32).reshape(nlay, N, H, C).transpose(0, 3, 2, 1)).astype(bf16)

    in_maps = []
    for cix in range(ncores):
        sl = slice(cix * NPC, (cix + 1) * NPC)
        in_maps.append({
            "featT": np.ascontiguousarray(
                featT[:, sl].reshape(KT0, 128, NPC).transpose(1, 0, 2)),
            "w0a": w0a_tiled,
            "wra": wra,
            "wb": wb,
            "at": np.ascontiguousarray(at_all[:, :, sl]),
            "wp0": np.ascontiguousarray(wp0r[:, :, sl]),
            "wpr": np.ascontiguousarray(wprr[:, :, :, sl]),
            "aff": aff,
        })
    return in_maps


def host_bias(inputs):
    inp = {k: np.asarray(v) for k, v in inputs.items()}
    return (inp["bp0"] + inp["bpR"].sum(axis=0)).astype(np.float32).reshape(1, C)


_CACHE = {}


def _get_program():
    if "nc" not in _CACHE:
        _CACHE["nc"] = build_program()
    return _CACHE["nc"]


def kernel(**inputs):
    nc = _get_program()
    in_maps = prep_inputs(inputs)
    res = run_bass_kernel_spmd(nc, in_maps, list(range(NCORES)))
    total = np.zeros((1, C), np.float32)
    for r in res.results:
        total += np.asarray(r["score"], np.float32).reshape(1, C)
    return (total + host_bias(inputs)).astype(np.float32)
